# revision 3
# baseline (speedup 1.0000x reference)
"""Trainium2 Bass kernel v3 for nn_MinamoTopoModel (3-layer GAT + mean-pool + FC).

Strategy (8 NeuronCores, SPMD), restructured for overlap:
  - L1 uses matmul linearity: out = (sum_e alpha_e x0[src]) @ W1, so only the
    128-wide x0 rows (+16 tail cols) are exchanged (6MB AllGather instead of
    85MB of post-dense features).  Per-head weighted input aggregation
    (8x128 wide) with a post-aggregation block-diagonal dense.
  - Single-pass edge loops: one full-row indirect gather per 128-edge tile;
    attention w from gathered es + one-hot-expanded ed; denominator as an
    appended matmul column; the per-head weighting and epilogue 1/den scaling
    are single stride-0-broadcast DVE ops.
  - AllGathers are chunked (8 x 320 rows) into a Local DRAM tensor (Shared
    would need a single writer) and triggered as soon as the dense blocks
    producing each chunk finish, so collectives overlap the edge loop.
  - The next layer's dense stage is software-pipelined INTO the following
    block's edge-tile emission (PIPE), keeping the PE fed during gather waits.
  - Activations stay in SBUF between layers (PE-transposed per block straight
    into the next dense's lhsT layout; no DRAM round-trip).
"""

import math
import os
from contextlib import ExitStack

import numpy as np
import ml_dtypes

import concourse.bass as bass
import concourse.bacc as bacc
import concourse.mybir as mybir
import concourse.tile as tile
from concourse.bass import IndirectOffsetOnAxis
from concourse.masks import make_identity

BF16 = mybir.dt.bfloat16
F32 = mybir.dt.float32
I32 = mybir.dt.int32
OP = mybir.AluOpType
ACT_EXP = mybir.ActivationFunctionType.Exp
ACT_COPY = mybir.ActivationFunctionType.Copy

SLOPE = 0.2
G = 16
NCORES = 8
NCHUNK = 8

bf16np = ml_dtypes.bfloat16
f8np = ml_dtypes.float8_e4m3fn
F8 = mybir.dt.float8e4
FSC = 32.0   # fp8 feature storage scale (descale folded into edge weights)


# --------------------------------------------------------------------------
# Host-side preprocessing
# --------------------------------------------------------------------------

def _pack_blocks(deg, sizes, cap):
    order = np.argsort(-deg, kind="stable")
    nb = len(sizes)
    loads = np.zeros(nb, dtype=np.int64)
    cnts = np.zeros(nb, dtype=np.int64)
    bins = [[] for _ in range(nb)]
    for d in order:
        g = deg[d]
        placed = False
        for i in np.argsort(loads, kind="stable"):
            if cnts[i] < sizes[i] and loads[i] + g <= cap:
                loads[i] += g
                cnts[i] += 1
                bins[i].append(d)
                placed = True
                break
        if not placed:
            return None
    return bins


def preprocess(x, edge_index, batch, cfg):
    N, NLOC, NB = cfg["N"], cfg["NLOC"], cfg["NB"]
    sizes = cfg["sizes"]
    NROWS = cfg["NROWS"]
    CH = cfg["CH"]
    ei = np.asarray(edge_index).astype(np.int64)
    bat = np.asarray(batch).astype(np.int64)
    x = np.asarray(x, dtype=np.float32)
    loop = np.arange(N, dtype=np.int64)
    src = np.concatenate([ei[0], loop])
    dst = np.concatenate([ei[1], loop])

    deg = np.bincount(dst, minlength=N)

    avg_cap = deg.reshape(NCORES, NLOC).sum(1).max() / NB
    T = max(1, math.ceil(avg_cap / 128))
    all_bins = None
    while True:
        cap = (T + 2) * 128
        all_bins = []
        ok = True
        for c in range(NCORES):
            bins = _pack_blocks(deg[c * NLOC:(c + 1) * NLOC], sizes, cap)
            if bins is None:
                ok = False
                break
            all_bins.append(bins)
        if ok:
            break
        T += 1
        assert T <= 64, "block packing failed"
    loads = np.zeros((NCORES, NB), dtype=np.int64)
    for c in range(NCORES):
        for b, bl in enumerate(all_bins[c]):
            loads[c, b] = deg[np.asarray(bl, dtype=np.int64) + c * NLOC].sum()
    tiles = np.maximum(1, np.ceil(loads.max(0) / 128).astype(int))
    offs = np.concatenate([[0], np.cumsum(tiles)])
    cfg = dict(cfg)
    cfg["tiles"] = [int(t) for t in tiles]
    cfg["offs"] = [int(o) for o in offs]
    TOT = int(offs[-1])
    cfg["TOT"] = TOT

    # per core: orig local id -> permuted local id (block-packed)
    perm = np.full((NCORES, NLOC), -1, dtype=np.int64)
    for c in range(NCORES):
        for b, bl in enumerate(all_bins[c]):
            for j, d in enumerate(bl):
                perm[c, d] = b * 128 + j
    assert (perm >= 0).all()
    owner = np.arange(N) // NLOC
    nl_of = perm[owner, np.arange(N) % NLOC]
    # chunk-major AllGather layout (L2/L3): (nl//CH)*(8*CH) + owner*CH + nl%CH
    g2r = (nl_of // CH) * (NCORES * CH) + owner * CH + (nl_of % CH)
    # core-major layout (single aug0 AllGather): owner*NROWS + nl
    g2r1 = owner * NROWS + nl_of

    ecore = dst // NLOC
    metas, ohs, ohts, gone, xT = [], [], [], [], []
    for c in range(NCORES):
        mrow = np.zeros((TOT * 128,), dtype=np.int32)
        mrow1 = np.zeros((TOT * 128,), dtype=np.int32)
        ohf = np.zeros((TOT * 128, 128), dtype=f8np)
        sel = np.nonzero(ecore == c)[0]
        nl = perm[c, dst[sel] - c * NLOC]
        blk = nl // 128
        slot = nl % 128
        rows = g2r[src[sel]]
        rows1 = g2r1[src[sel]]
        order = np.lexsort((rows, blk))
        blk, slot = blk[order], slot[order]
        rows, rows1 = rows[order], rows1[order]
        pos = 0
        for b in range(NB):
            cnt = int((blk == b).sum())
            base = offs[b] * 128
            assert cnt <= tiles[b] * 128
            mrow[base:base + cnt] = rows[pos:pos + cnt]
            mrow1[base:base + cnt] = rows1[pos:pos + cnt]
            ohf[np.arange(base, base + cnt), slot[pos:pos + cnt]] = 1.0
            pos += cnt
        metas.append(np.stack([
            np.ascontiguousarray(mrow1.reshape(TOT, 128).T),
            np.ascontiguousarray(mrow.reshape(TOT, 128).T)]).astype(np.int32))
        oh3 = ohf.reshape(TOT, 128, 128)
        ohs.append(np.ascontiguousarray(oh3.transpose(1, 0, 2)))
        ohts.append(np.ascontiguousarray(oh3.transpose(2, 0, 1)))

        inv = np.empty(NLOC, dtype=np.int64)
        inv[perm[c]] = np.arange(NLOC)
        orig = inv + c * NLOC
        go = np.zeros((NROWS, G), dtype=bf16np)
        go[np.arange(NLOC), bat[orig]] = 1.0
        gone.append(go)

        xt = np.zeros((x.shape[1], NROWS), dtype=np.float32)
        xt[:, :NLOC] = x[orig].T
        xT.append(xt)

    cnts = np.bincount(bat, minlength=G).astype(np.float32)
    crecip = (1.0 / np.maximum(cnts, 1.0)).reshape(G, 1).astype(np.float32)

    return cfg, metas, ohs, ohts, gone, xT, crecip


def _fold_tails(W, a_s, a_d, heads, ch):
    """[k, 16] = [va_s pad8 | va_d pad8], va[k,h] = sum_c W[k,h*ch+c]*a[h,c]."""
    W = np.asarray(W, dtype=np.float64)
    a_s = np.asarray(a_s, dtype=np.float64).reshape(heads, ch)
    a_d = np.asarray(a_d, dtype=np.float64).reshape(heads, ch)
    k = W.shape[0]
    W3 = W.reshape(k, heads, ch)
    va_s = np.einsum("khc,hc->kh", W3, a_s)
    va_d = np.einsum("khc,hc->kh", W3, a_d)
    pad = np.zeros((k, 8 - heads), dtype=np.float64)
    return np.concatenate([va_s, pad, va_d, pad], axis=1).astype(np.float32)


# --------------------------------------------------------------------------
# Bass program
# --------------------------------------------------------------------------

def build_program(cfg):
    N, NLOC, NB = cfg["N"], cfg["NLOC"], cfg["NB"]
    sizes = cfg["sizes"]
    btiles, offs = cfg["tiles"], cfg["offs"]
    TMAX = max(btiles)
    TOT = cfg["TOT"]
    NROWS = cfg["NROWS"]
    CH = cfg["CH"]
    TILEF, EMB, HID, OUT, FEAT, HEADS = 32, 128, 256, 512, 512, 8
    D1, D2, D3 = HEADS * HID, HEADS * HID, OUT   # 2048, 2048, 512
    W1AUG, W2AUG, W3AUG = EMB + 16, D2 + 32, D3 + 32   # 144, 2080, 544
    KT2 = D1 // 128   # 16
    # AG chunk c (rows [c*CH, (c+1)*CH)) is ready after this dense block:
    trig_blk = [math.ceil((c + 1) * CH / 128) - 1 for c in range(NCHUNK)]

    nc = bacc.Bacc(num_devices=NCORES)

    # ---------------- I/O ----------------
    xT_in = nc.dram_tensor("xT_in", [TILEF, NROWS], F32, kind="ExternalInput")
    meta_in = nc.dram_tensor("meta", [2, 128, TOT], I32, kind="ExternalInput")
    oh_in = nc.dram_tensor("oh", [128, TOT, 128], F8, kind="ExternalInput")
    oht_in = nc.dram_tensor("oht", [128, TOT, 128], F8, kind="ExternalInput")
    gone_in = nc.dram_tensor("gone", [NROWS, G], BF16, kind="ExternalInput")
    crecip_in = nc.dram_tensor("crecip", [G, 1], F32, kind="ExternalInput")
    W0_in = nc.dram_tensor("W0", [TILEF, EMB], F32, kind="ExternalInput")
    b0_in = nc.dram_tensor("b0", [1, EMB], BF16, kind="ExternalInput")
    Wa1_in = nc.dram_tensor("Wa1", [EMB, 16], BF16, kind="ExternalInput")
    W1_in = nc.dram_tensor("W1", [EMB, D1], BF16, kind="ExternalInput")
    W2_in = nc.dram_tensor("W2", [D1, 2064], BF16, kind="ExternalInput")
    W3_in = nc.dram_tensor("W3", [D2, 528], BF16, kind="ExternalInput")
    Wf_in = nc.dram_tensor("Wf", [OUT, FEAT], BF16, kind="ExternalInput")
    bf_in = nc.dram_tensor("bfc", [1, FEAT], F32, kind="ExternalInput")
    out_ext = nc.dram_tensor("out", [G, FEAT], F32, kind="ExternalOutput")

    with tile.TileContext(nc) as tc, ExitStack() as ctx:
        dram = ctx.enter_context(tc.tile_pool(name="dram", bufs=1, space="DRAM"))
        cpool = ctx.enter_context(tc.tile_pool(name="consts", bufs=1))
        wpool = ctx.enter_context(tc.tile_pool(name="weights", bufs=1))
        sb = ctx.enter_context(tc.tile_pool(name="work", bufs=2))
        gfp = ctx.enter_context(tc.tile_pool(name="gfp", bufs=3))
        pacc = ctx.enter_context(tc.tile_pool(name="pacc", bufs=1, space="PSUM"))
        pdps = ctx.enter_context(tc.tile_pool(name="pdps", bufs=1, space="PSUM"))
        psm = ctx.enter_context(tc.tile_pool(name="psm", bufs=1, space="PSUM"))
        pden = ctx.enter_context(tc.tile_pool(name="pden", bufs=1, space="PSUM"))

        # ------------- internal DRAM -------------
        aug0_l = dram.tile([NROWS, W1AUG], BF16, name="aug0l")
        aug0_f = dram.tile([NCORES, NROWS, W1AUG], BF16, name="aug0f",
                           addr_space="Shared")
        aug2_l = dram.tile([NROWS, W2AUG], F8, name="aug2l")
        aug2_f = dram.tile([NCHUNK, NCORES, CH, W2AUG], F8, name="aug2f")
        aug3_l = dram.tile([NROWS, W3AUG], F8, name="aug3l")
        aug3_f = dram.tile([NCHUNK, NCORES, CH, W3AUG], F8, name="aug3f")
        pool_in = dram.tile([G, FEAT], F32, name="pool_in")
        pool_out = dram.tile([G, FEAT], F32, name="pool_out",
                             addr_space="Shared")
        aug0_rows = aug0_f[:, :, :].rearrange("c r w -> (c r) w")
        aug2_rows = aug2_f[:, :, :, :].rearrange("k c r w -> (k c r) w")
        aug3_rows = aug3_f[:, :, :, :].rearrange("k c r w -> (k c r) w")

        # ------------- constants + weights -------------
        ident = cpool.tile([128, 128], BF16, name="ident")
        make_identity(nc, ident[:])
        meta_sb = cpool.tile([128, 2, TOT], I32, name="metasb")
        nc.sync.dma_start(meta_sb[:],
                          meta_in[:, :, :].rearrange("a p t -> p a t"))
        oh_sb = cpool.tile([128, TOT, 128], F8, name="ohsb")
        tchunk = (TOT + 7) // 8
        for i in range(8):
            t0 = i * tchunk
            t1 = min(TOT, t0 + tchunk)
            if t0 < t1:
                nc.sync.dma_start(oh_sb[:, t0:t1, :], oh_in[:, t0:t1, :])

        W0_sb = wpool.tile([TILEF, EMB], F32, name="W0sb")
        nc.sync.dma_start(W0_sb[:], W0_in[:, :])
        b0_sb = wpool.tile([128, EMB], BF16, name="b0sb")
        nc.sync.dma_start(b0_sb[:], b0_in[:, :].to_broadcast([128, EMB]))
        Wa1_sb = wpool.tile([EMB, 16], BF16, name="Wa1sb")
        nc.sync.dma_start(Wa1_sb[:], Wa1_in[:, :])
        W1_sb = wpool.tile([EMB, D1], BF16, name="W1sb")
        nc.sync.dma_start(W1_sb[:], W1_in[:, :])
        bf_sb = wpool.tile([G, FEAT], F32, name="bfsb")
        nc.sync.dma_start(bf_sb[:], bf_in[:, :].to_broadcast([G, FEAT]))
        crecip_sb = wpool.tile([G, 1], F32, name="crecipsb")
        nc.sync.dma_start(crecip_sb[:], crecip_in[:, :])
        gone_sb = wpool.tile([128, NB, G], BF16, name="gonesb")
        nc.sync.dma_start(
            gone_sb[:], gone_in[:, :].rearrange("(b p) g -> p b g", p=128))

        W2b = []
        for k in range(KT2):
            t = wpool.tile([128, 2064], BF16, name=f"W2k{k}", tag=f"Wp{k}",
                           bufs=1)
            nc.sync.dma_start(t[:], W2_in[k * 128:(k + 1) * 128, :])
            W2b.append(t)
        W3b = []
        for k in range(KT2):
            t = wpool.tile([128, 528], BF16, name=f"W3k{k}", tag=f"Wq{k}",
                           bufs=1)
            nc.sync.dma_start(t[:], W3_in[k * 128:(k + 1) * 128, :])
            W3b.append(t)

        def leaky(out_ap, in_ap, tmp_tile):
            """out = max(in, 0.2*in): scalar does the scaled copy, vector max."""
            nc.scalar.activation(tmp_tile, in_ap, ACT_COPY, scale=SLOPE)
            nc.vector.tensor_tensor(out=out_ap, in0=in_ap, in1=tmp_tile,
                                    op=OP.max)

        def ag_chunk(chk, loc, ful):
            c0 = chk * CH
            nc.gpsimd.collective_compute(
                "AllGather", OP.bypass,
                replica_groups=[list(range(NCORES))],
                ins=[loc[c0:c0 + CH, :].opt()],
                outs=[ful[chk].opt()])

        # deferred-emission pipeline: dense work of block b is emitted
        # interleaved into block b+1's edge-tile stream
        PIPE = []

        def drain(k=None):
            n = len(PIPE) if k is None else min(k, len(PIPE))
            for _ in range(n):
                PIPE.pop(0)()

        # ============ L0: x0 = leaky(x @ W0 + b0) + tails, to aug0 ============
        for m in range(NB):
            rows = sizes[m]
            ms = slice(m * 128, (m + 1) * 128)
            lhs0 = sb.tile([TILEF, 128], F32, name="lhs0", tag="lhs0", bufs=2)
            nc.sync.dma_start(lhs0[:], xT_in[:, ms])
            dps0 = pdps.tile([128, 1024], F32, name="dps0", tag="dps")
            nc.tensor.matmul(dps0[:, 0:EMB], lhsT=lhs0[:], rhs=W0_sb[:],
                             start=True, stop=True)
            hraw = sb.tile([128, EMB], BF16, name="hraw0", tag="x0h", bufs=2)
            nc.vector.tensor_tensor(out=hraw[:], in0=dps0[:, 0:EMB],
                                    in1=b0_sb[:], op=OP.add)
            x0r = sb.tile([128, W1AUG], BF16, name="x0r", tag="x0r", bufs=2)
            tmp0 = sb.tile([128, EMB], BF16, name="tmp0", tag="x0t", bufs=2)
            leaky(x0r[:, 0:EMB], hraw[:], tmp0[:])
            xt_ps = pden.tile([128, 128], BF16, name="xtps", tag="den")
            nc.tensor.transpose(xt_ps[:], x0r[:, 0:EMB], ident[:])
            xt_sb = sb.tile([128, 128], BF16, name="xtsb", tag="xtsb", bufs=2)
            nc.vector.tensor_copy(xt_sb[:], xt_ps[:])
            tl_ps = pden.tile([128, 16], F32, name="tlps0", tag="den")
            nc.tensor.matmul(tl_ps[:], lhsT=xt_sb[:], rhs=Wa1_sb[:],
                             start=True, stop=True)
            nc.scalar.activation(x0r[:, EMB:W1AUG], tl_ps[:], ACT_COPY)
            nc.sync.dma_start(aug0_l[m * 128:m * 128 + rows, :], x0r[:rows, :])
            if m == NB - 1:
                nc.gpsimd.collective_compute(
                    "AllGather", OP.bypass,
                    replica_groups=[list(range(NCORES))],
                    ins=[aug0_l[:, :].opt()],
                    outs=[aug0_f[:, :, :].opt()])

        # ============ generic edge phase ============
        def edge_block(b, aug_rows_v, aug_l, gw, featw, fdt, nh, agg_w,
                       dense_fn, mi):
            """One dst block: gather tiles, attention, weighted scatter-add.
            gw = gathered row width (elements of dtype fdt); featw = feature
            element count.  For fp8 layers (fdt=F8) the 16 bf16 attention
            tails live bitcast in the last 32 f8 bytes and the stored
            features carry a factor FSC that is undone in the edge weight.
            dense_fn(b, acc, rd) emits the epilogue inline and queues the
            next-layer dense for this block onto PIPE."""
            rows = sizes[b]
            r0 = b * 128
            o = offs[b]
            TB = btiles[b]
            ow = agg_w // nh
            fp8f = fdt is F8
            expand = featw != agg_w   # L1: broadcast x0 into 8 head slices
            pace = max(1, -(-len(PIPE) // max(1, TB - 1)))
            ed_blk = sb.tile([128, 8], BF16, name=f"ed{b}", tag="edblk",
                             bufs=2)
            if rows < 128:
                nc.vector.memset(ed_blk[:], 0.0)
            if fp8f:
                ed_src = aug_l[r0:r0 + rows,
                               featw + 16:featw + 32].bitcast(BF16)
            else:
                ed_src = aug_l[r0:r0 + rows, featw + 8:featw + 16]
            nc.sync.dma_start(ed_blk[:rows, :], ed_src)
            oht_sl = sb.tile([128, TMAX, 128], F8, name=f"oht{b}",
                             tag="ohts", bufs=2)
            nc.sync.dma_start(oht_sl[:, 0:TB, :], oht_in[:, o:o + TB, :])
            acc = pacc.tile([128, 2048], F32, name=f"acc{b}", tag="acc")
            den_ps = pden.tile([128, 8], F32, name=f"den{b}", tag="den")
            edpe = psm.tile([128, TMAX, 8], F32, name=f"edpe{b}", tag="sm")
            for t in range(TB):
                ti = o + t
                gf = gfp.tile([128, gw], fdt, name=f"gf{b}", tag="gf",
                              bufs=6)
                nc.gpsimd.indirect_dma_start(
                    out=gf[:], out_offset=None,
                    in_=aug_rows_v,
                    in_offset=IndirectOffsetOnAxis(
                        ap=meta_sb[:, mi, ti:ti + 1], axis=0))
                drain(pace)
                if fp8f:
                    es_ap = gf[:, featw:featw + 16].bitcast(BF16)[:, 0:nh]
                else:
                    es_ap = gf[:, featw:featw + nh]
                nc.tensor.matmul(edpe[:, t, 0:nh], lhsT=oht_sl[:, t, :],
                                 rhs=ed_blk[:, 0:nh], start=True, stop=True)
                e_t = sb.tile([128, 8], F32, name=f"et{b}", tag="et", bufs=3)
                nc.vector.tensor_tensor(out=e_t[:, 0:nh], in0=es_ap,
                                        in1=edpe[:, t, 0:nh], op=OP.add)
                w1 = sb.tile([128, 8], F32, name=f"w1{b}", tag="w1", bufs=3)
                w2 = sb.tile([128, 8], F32, name=f"w2{b}", tag="w2", bufs=3)
                nc.scalar.activation(w1[:, 0:nh], e_t[:, 0:nh], ACT_EXP)
                nc.scalar.activation(w2[:, 0:nh], e_t[:, 0:nh], ACT_EXP,
                                     scale=SLOPE)
                wv = sb.tile([128, 8], F32, name=f"wv{b}", tag="wv", bufs=3)
                nc.vector.tensor_tensor(out=wv[:, 0:nh], in0=w1[:, 0:nh],
                                        in1=w2[:, 0:nh], op=OP.max)
                fwt = sb.tile([128, 8], BF16, name=f"fwt{b}", tag="fwt",
                              bufs=3)
                nc.scalar.activation(fwt[:, 0:nh], wv[:, 0:nh], ACT_COPY)
                if fp8f:
                    # weight the one-hot columns (128-wide) instead of the
                    # features: ohw = oh * (w / FSC); scatter per head with
                    # ohw stationary and the raw fp8 features streaming.
                    wf = sb.tile([128, 8], BF16, name=f"wf{b}", tag="wfs",
                                 bufs=3)
                    nc.scalar.activation(wf[:, 0:nh], wv[:, 0:nh], ACT_COPY,
                                         scale=1.0 / FSC)
                    ohw = sb.tile([128, nh, 128], BF16, name=f"ohw{b}",
                                  tag="ohw", bufs=3)
                    nc.vector.tensor_tensor(
                        out=ohw[:],
                        in0=oh_sb[:, ti, :].rearrange(
                            "p (a c) -> p a c", a=1).to_broadcast(
                            [128, nh, 128]),
                        in1=wf[:, 0:nh][:, :, None].to_broadcast(
                            [128, nh, 128]),
                        op=OP.mult)
                    hpb = max(1, 512 // ow)   # heads per psum bank
                    for h in range(nh):
                        nc.tensor.matmul(acc[:, h * ow:(h + 1) * ow],
                                         lhsT=ohw[:, h, :],
                                         rhs=gf[:, h * ow:(h + 1) * ow],
                                         start=(t == 0 and h % hpb == 0),
                                         stop=(t == TB - 1
                                               and h % hpb == hpb - 1),
                                         skip_group_check=True)
                else:
                    fw = sb.tile([128, agg_w], BF16, name=f"fw{b}", tag="fw",
                                 bufs=3)
                    in0 = gf[:, 0:featw].rearrange(
                        "p (a c) -> p a c", a=1).to_broadcast(
                        [128, nh, ow])
                    nc.vector.tensor_tensor(
                        out=fw[:, 0:agg_w].rearrange("p (h c) -> p h c",
                                                     h=nh),
                        in0=in0,
                        in1=fwt[:, 0:nh][:, :, None].to_broadcast(
                            [128, nh, ow]),
                        op=OP.mult)
                    for j in range(0, agg_w, 512):
                        nc.tensor.matmul(acc[:, j:j + 512],
                                         lhsT=oh_sb[:, ti, :],
                                         rhs=fw[:, j:j + 512],
                                         start=(t == 0), stop=(t == TB - 1))
                nc.tensor.matmul(den_ps[:, 0:nh], lhsT=oh_sb[:, ti, :],
                                 rhs=fwt[:, 0:nh],
                                 start=(t == 0), stop=(t == TB - 1))
            drain()
            den_sb = sb.tile([128, 8], F32, name=f"dsb{b}", tag="dsb", bufs=2)
            nc.vector.tensor_scalar_add(den_sb[:, 0:nh], den_ps[:, 0:nh],
                                        1e-16)
            rd = sb.tile([128, 8], F32, name=f"rd{b}", tag="rd", bufs=2)
            nc.vector.reciprocal(rd[:, 0:nh], den_sb[:, 0:nh])
            dense_fn(b, acc, rd)

        # ---- L1 per-block epilogue (inline) + queued L2 dense ----
        def l1_dense(b, acc, rd):
            rows = sizes[b]
            r0 = b * 128
            aggs = sb.tile([128, 1024], BF16, name=f"aggs{b}", tag="rowm",
                           bufs=2)
            nc.vector.tensor_tensor(
                out=aggs[:].rearrange("p (h c) -> p h c", h=HEADS),
                in0=acc[:, 0:1024].rearrange("p (h c) -> p h c", h=HEADS),
                in1=rd[:, 0:HEADS][:, :, None].to_broadcast(
                    [128, HEADS, 128]),
                op=OP.mult)
            aggT_ps = pden.tile([128, 8, 128], BF16, name=f"aggTp{b}",
                                tag="den")
            for h in range(HEADS):
                nc.tensor.transpose(aggT_ps[:, h, :],
                                    aggs[:, h * 128:(h + 1) * 128], ident[:])
            aggT = sb.tile([128, 8, 128], BF16, name=f"aggT{b}", tag="aggT",
                           bufs=2)
            nc.vector.tensor_copy(aggT[:], aggT_ps[:])
            x1T = sb.tile([128, KT2, 128], BF16, name=f"x1T{b}", tag="xT",
                          bufs=2)
            for r in range(2):
                for j in range(8):
                    c = r * 8 + j
                    nc.tensor.matmul(
                        acc[:, 1024 + j * 128:1024 + (j + 1) * 128],
                        lhsT=W1_sb[:, c * 128:(c + 1) * 128],
                        rhs=aggT[:, c // 2, :],
                        start=(j % 4 == 0), stop=(j % 4 == 3),
                        skip_group_check=True)
                part = x1T[:, r * 8:(r + 1) * 8, :].rearrange(
                    "p a b -> p (a b)")
                tmpl = sb.tile([128, 1024], BF16, name=f"tl{b}", tag="tmpl",
                               bufs=2)
                leaky(part, acc[:, 1024:2048], tmpl[:])
            # ---- queued L2 dense for this block ----
            h_sb = sb.tile([128, W2AUG], F8, name=f"h2{b}", tag="hsb",
                           bufs=2)
            st = {}

            def mk_mm(half, k0, k1):
                def go():
                    if k0 == 0:
                        st["dps"] = pdps.tile([128, 1024], F32, name=f"d2{b}",
                                              tag="dps")
                    dps = st["dps"]
                    for k in range(k0, k1):
                        for cc in range(2):
                            c0 = half * 1024 + cc * 512
                            nc.tensor.matmul(
                                dps[:, cc * 512:(cc + 1) * 512],
                                lhsT=x1T[:, k, :],
                                rhs=W2b[k][:, c0:c0 + 512],
                                start=(k == 0), stop=(k == KT2 - 1))
                return go

            def mk_cp(half):
                def go():
                    dps = st["dps"]
                    nc.vector.tensor_scalar_mul(
                        h_sb[:, half * 1024:half * 1024 + 512],
                        dps[:, 0:512], FSC)
                    nc.scalar.activation(
                        h_sb[:, half * 1024 + 512:(half + 1) * 1024],
                        dps[:, 512:1024], ACT_COPY, scale=FSC)
                return go

            def mk_tails():
                def go():
                    tps = pdps.tile([128, 16], F32, name=f"t2{b}", tag="dps")
                    for k in range(KT2):
                        nc.tensor.matmul(tps[:], lhsT=x1T[:, k, :],
                                         rhs=W2b[k][:, D2:D2 + 16],
                                         start=(k == 0), stop=(k == KT2 - 1))
                    nc.scalar.activation(
                        h_sb[:, D2:W2AUG].bitcast(BF16), tps[:], ACT_COPY)
                return go

            def mk_dma():
                def go():
                    nc.sync.dma_start(aug2_l[r0:r0 + rows, :], h_sb[:rows, :])
                    for c in range(NCHUNK):
                        if trig_blk[c] == b:
                            ag_chunk(c, aug2_l, aug2_f)
                return go

            for half in range(2):
                PIPE.append(mk_mm(half, 0, 8))
                PIPE.append(mk_mm(half, 8, KT2))
                PIPE.append(mk_cp(half))
            PIPE.append(mk_tails())
            PIPE.append(mk_dma())

        for b in range(NB):
            edge_block(b, aug0_rows, aug0_l, W1AUG, EMB, BF16, HEADS,
                       1024, l1_dense, 0)
        drain()

        # ---- L2 per-block epilogue (inline) + queued L3 dense ----
        def l2_dense(b, acc, rd):
            rows = sizes[b]
            r0 = b * 128
            xs = sb.tile([128, 2048], BF16, name=f"xs{b}", tag="rowm", bufs=2)
            nc.vector.tensor_tensor(
                out=xs[:].rearrange("p (h c) -> p h c", h=HEADS),
                in0=acc[:, 0:2048].rearrange("p (h c) -> p h c", h=HEADS),
                in1=rd[:, 0:HEADS][:, :, None].to_broadcast(
                    [128, HEADS, 256]),
                op=OP.mult)
            x2 = sb.tile([128, 2048], BF16, name=f"x2{b}", tag="rowm2",
                         bufs=2)
            tmpl = sb.tile([128, 2048], BF16, name=f"tl2{b}", tag="tmpl",
                           bufs=2)
            leaky(x2[:], xs[:], tmpl[:])
            x2T = sb.tile([128, KT2, 128], BF16, name=f"x2T{b}", tag="xT",
                          bufs=2)
            for r in range(2):
                xtp = pden.tile([128, 8, 128], BF16, name=f"xtp{b}",
                                tag="den")
                for j in range(8):
                    c = r * 8 + j
                    nc.tensor.transpose(xtp[:, j, :],
                                        x2[:, c * 128:(c + 1) * 128],
                                        ident[:])
                nc.vector.tensor_copy(x2T[:, r * 8:(r + 1) * 8, :], xtp[:])
            # ---- queued L3 dense (tails fused into one 528-col pass) ----
            h3 = sb.tile([128, W3AUG], F8, name=f"h3{b}", tag="hsb", bufs=2)
            st = {}

            def mk_mm(k0, k1):
                def go():
                    if k0 == 0:
                        st["dps"] = pdps.tile([128, 1024], F32, name=f"d3{b}",
                                              tag="dps")
                    dps = st["dps"]
                    for k in range(k0, k1):
                        nc.tensor.matmul(dps[:, 0:512], lhsT=x2T[:, k, :],
                                         rhs=W3b[k][:, 0:512],
                                         start=(k == 0), stop=(k == KT2 - 1))
                        nc.tensor.matmul(dps[:, 512:512 + 16],
                                         lhsT=x2T[:, k, :],
                                         rhs=W3b[k][:, 512:512 + 16],
                                         start=(k == 0), stop=(k == KT2 - 1))
                return go

            def mk_cp():
                def go():
                    dps = st["dps"]
                    nc.vector.tensor_scalar_mul(h3[:, 0:512], dps[:, 0:512],
                                                FSC)
                    nc.scalar.activation(h3[:, 512:W3AUG].bitcast(BF16),
                                         dps[:, 512:512 + 16], ACT_COPY)
                return go

            def mk_dma():
                def go():
                    nc.sync.dma_start(aug3_l[r0:r0 + rows, :], h3[:rows, :])
                    for c in range(NCHUNK):
                        if trig_blk[c] == b:
                            ag_chunk(c, aug3_l, aug3_f)
                return go

            PIPE.append(mk_mm(0, 8))
            PIPE.append(mk_mm(8, KT2))
            PIPE.append(mk_cp())
            PIPE.append(mk_dma())

        for b in range(NB):
            edge_block(b, aug2_rows, aug2_l, W2AUG, D2, F8, HEADS,
                       2048, l2_dense, 1)
        drain()

        # ---- L3 edge phase + pooling ----
        pool_ps = pdps.tile([G, FEAT], F32, name="poolps", tag="dps")

        def l3_dense(b, acc, rd):
            xs = sb.tile([128, 512], BF16, name=f"xs3{b}", tag="rowm",
                         bufs=2)
            nc.vector.tensor_scalar_mul(xs[:], acc[:, 0:512], rd[:, 0:1])
            x3 = sb.tile([128, 512], BF16, name=f"x3{b}", tag="rowm2", bufs=2)
            tmpl = sb.tile([128, 512], BF16, name=f"tl3{b}", tag="tmpl",
                           bufs=2)
            leaky(x3[:], xs[:], tmpl[:])
            nc.tensor.matmul(pool_ps[:], lhsT=gone_sb[:, b, :], rhs=x3[:],
                             start=(b == 0), stop=(b == NB - 1))

        for b in range(NB):
            edge_block(b, aug3_rows, aug3_l, W3AUG, D3, F8, 1,
                       512, l3_dense, 1)

        # ---- pooling reduce + FC ----
        psum_sb = sb.tile([G, FEAT], F32, name="psum_sb", tag="fc16", bufs=3)
        nc.vector.tensor_copy(psum_sb[:], pool_ps[:])
        nc.sync.dma_start(pool_in[:, :], psum_sb[:])
        nc.gpsimd.collective_compute(
            "AllReduce", OP.add, replica_groups=[list(range(NCORES))],
            ins=[pool_in[:, :].opt()], outs=[pool_out[:, :].opt()])
        psum_all = sb.tile([G, FEAT], F32, name="psum_all", tag="fc16", bufs=3)
        nc.sync.dma_start(psum_all[:], pool_out[:, :])
        mean_f = sb.tile([G, FEAT], F32, name="mean_f", tag="fc16", bufs=3)
        nc.vector.tensor_scalar_mul(mean_f[:], psum_all[:], crecip_sb[:, 0:1])
        mean_bf = sb.tile([G, FEAT], BF16, name="mean_bf", tag="fc16", bufs=3)
        nc.vector.tensor_copy(mean_bf[:], mean_f[:])
        Wf_sb = []
        for k in range(OUT // 128):
            t = wpool.tile([128, FEAT], BF16, name=f"Wfk{k}", tag=f"Wp{k}",
                           bufs=1)
            nc.sync.dma_start(t[:], Wf_in[k * 128:(k + 1) * 128, :])
            Wf_sb.append(t)
        fc_ps = pdps.tile([G, 512], F32, name="fcps", tag="dps")
        for k in range(OUT // 128):
            mT_ps = pden.tile([128, G], BF16, name="mTps", tag="den")
            nc.tensor.transpose(mT_ps[:], mean_bf[:, k * 128:(k + 1) * 128],
                                ident[:G, :G])
            mT = sb.tile([128, G], BF16, name="mT", tag="mT", bufs=2)
            nc.vector.tensor_copy(mT[:], mT_ps[:])
            nc.tensor.matmul(fc_ps[:], lhsT=mT[:], rhs=Wf_sb[k][:],
                             start=(k == 0), stop=(k == OUT // 128 - 1))
        fc_raw = sb.tile([G, FEAT], F32, name="fc_raw", tag="fc16", bufs=3)
        nc.vector.tensor_tensor(out=fc_raw[:], in0=fc_ps[:], in1=bf_sb[:],
                                op=OP.add)
        fc_t = sb.tile([G, FEAT], F32, name="fc_t", tag="fc16", bufs=3)
        fc_o = sb.tile([G, FEAT], F32, name="fc_o", tag="fc16", bufs=3)
        leaky(fc_o[:], fc_raw[:], fc_t[:])
        nc.sync.dma_start(out_ext[:, :], fc_o[:])

    nc.finalize()
    return nc


# --------------------------------------------------------------------------
# Entry point
# --------------------------------------------------------------------------

def make_cfg(N):
    NLOC = N // NCORES
    NB = (NLOC + 127) // 128
    sizes = [128] * (NLOC // 128) + ([NLOC % 128] if NLOC % 128 else [])
    NROWS = NB * 128
    assert NROWS % NCHUNK == 0
    return {"N": N, "NLOC": NLOC, "NB": NB, "sizes": sizes, "NROWS": NROWS,
            "CH": NROWS // NCHUNK}


def prepare_in_maps(inputs, cfg=None):
    x = np.asarray(inputs["x"], dtype=np.float32)
    N = x.shape[0]
    if cfg is None:
        cfg = make_cfg(N)
    cfg, metas, ohs, ohts, gone, xT, crecip = preprocess(
        x, inputs["edge_index"], inputs["batch"], cfg)

    def b16(a):
        return np.asarray(a, dtype=np.float32).astype(bf16np)

    for bn in ("b1", "b2", "b3"):
        assert np.abs(np.asarray(inputs[bn])).max() == 0.0

    W1 = np.asarray(inputs["W1"], np.float32)
    W2 = np.asarray(inputs["W2"], np.float32)
    W3 = np.asarray(inputs["W3"], np.float32)
    shared = {
        "W0": np.asarray(inputs["W0"], np.float32),
        "b0": b16(inputs["b0"]).reshape(1, -1),
        "Wa1": _fold_tails(W1, inputs["a1s"], inputs["a1d"], 8, 256)
        .astype(bf16np),
        "W1": b16(W1),
        "W2": np.concatenate(
            [W2, _fold_tails(W2, inputs["a2s"], inputs["a2d"], 8, 256)],
            axis=1).astype(bf16np),
        "W3": np.concatenate(
            [W3, _fold_tails(W3, inputs["a3s"], inputs["a3d"], 1, 512)],
            axis=1).astype(bf16np),
        "Wf": b16(inputs["Wf"]),
        "bfc": np.asarray(inputs["bf"], np.float32).reshape(1, -1),
        "crecip": crecip,
    }
    in_maps = []
    for c in range(NCORES):
        m = dict(shared)
        m["xT_in"] = xT[c]
        m["meta"] = metas[c]
        m["oh"] = ohs[c]
        m["oht"] = ohts[c]
        m["gone"] = gone[c]
        in_maps.append(m)
    return cfg, in_maps


_CACHE = {}


def _ensure_ntff_hook():
    import sys
    import types
    try:
        from antenv.axon_hooks import get_axon_ntff_profile_hook  # noqa: F401
        return
    except ImportError:
        pass
    try:
        import antenv
        from trn_agent_boot.trn_boot import _ntff_profile_via_ctypes
    except ImportError:
        return
    mod = types.ModuleType("antenv.axon_hooks")
    mod._hook = None
    mod.set_axon_ntff_profile_hook = lambda h: setattr(mod, "_hook", h)
    mod.get_axon_ntff_profile_hook = lambda: mod._hook
    sys.modules["antenv.axon_hooks"] = mod
    antenv.axon_hooks = mod
    try:
        mod._hook = _ntff_profile_via_ctypes("/opt/axon/libaxon_pjrt.so")
    except Exception:
        mod._hook = None


def kernel(**inputs) -> np.ndarray:
    from concourse.bass_utils import run_bass_kernel_spmd
    if os.environ.get("GNN_TRACE"):
        _ensure_ntff_hook()
    cfg, in_maps = prepare_in_maps(inputs)
    key = (cfg["N"], cfg["NB"], tuple(cfg["tiles"]))
    if key not in _CACHE:
        _CACHE[key] = build_program(cfg)
    nc = _CACHE[key]
    res = run_bass_kernel_spmd(nc, in_maps, core_ids=list(range(NCORES)),
                               trace=bool(os.environ.get("GNN_TRACE")))
    out = res.results[0]["out"]
    kernel.last_exec_time_ns = res.exec_time_ns
    kernel.last_results = res
    return np.asarray(out, dtype=np.float32)
